# revision 5
# baseline (speedup 1.0000x reference)
"""Causal self-attention TRN2 kernel (B=2, L=2048, D=1024, H=16, dh=64).

Sharding: 8 cores = 2 batches x 4 head-groups. Core c handles batch c//4 and
heads [4g..4g+3] where g = c % 4, as two head-pairs ("units").

Per-core device program (SPMD, same program all cores, different data):
  phase 1: qkvT = W_local.T @ x_b   (fp32r, contraction on partitions)
  phase 2: per unit (head-pair), per 512-wide q-block:
           S^T chunks [128k x 512q] via fp32r K=64 row-split matmul pairs,
           exp on ScalarE (PSUM->SBUF, bf16 out, scale=1/8, no max-sub),
           causal stair masks via bf16 DVE multiplies,
           AV (col-packed M=64 pair) + row-sums l (M=1 pair) in bf16,
           normalize by 1/l (DVE reciprocal + gpsimd partition_broadcast).
  phase 3: y_partial = outT.T @ w_out_local (fp32r), DMA to DRAM.
Host: y[b] = sum of the 4 partial outputs for batch b.
"""

import numpy as np
import ml_dtypes

import concourse.bass as bass
import concourse.mybir as mybir
from concourse import bacc
import concourse.tile as tile
from concourse.bass_utils import run_bass_kernel_spmd

F32 = mybir.dt.float32
F32R = mybir.dt.float32r
BF16 = mybir.dt.bfloat16
EXP = mybir.ActivationFunctionType.Exp

B, L, D = 2, 2048, 1024
H, DH = 16, 64
NCORES = 8
NQB = L // 512          # q-blocks per sequence (4)

_CACHE = {}
LAST_RESULT = None      # BassKernelResults of the most recent run (for test.py)


def _build():
    nc = bacc.Bacc("TRN2", target_bir_lowering=False, debug=False,
                   num_devices=NCORES)

    xt_d = nc.dram_tensor("xt", [D, L], F32R, kind="ExternalInput").ap()
    wqkv_d = nc.dram_tensor("wqkv", [D, 768], F32R, kind="ExternalInput").ap()
    wout_d = nc.dram_tensor("wout", [2, 128, 1024], F32R,
                            kind="ExternalInput").ap()
    masks_d = nc.dram_tensor("masks", [4, 128, 512], BF16,
                             kind="ExternalInput").ap()
    consts_d = nc.dram_tensor("consts", [128, 66], BF16,
                              kind="ExternalInput").ap()
    y_d = nc.dram_tensor("y", [L, D], F32, kind="ExternalOutput").ap()

    with tile.TileContext(nc) as tc:
        with tc.tile_pool(name="persist", bufs=1) as pp, \
             tc.tile_pool(name="work", bufs=1) as wp, \
             tc.tile_pool(name="psmm", bufs=2, space="PSUM") as ps_mm, \
             tc.tile_pool(name="psstage", bufs=2, space="PSUM") as ps_stage, \
             tc.tile_pool(name="psav", bufs=1, space="PSUM") as ps_av, \
             tc.tile_pool(name="psl", bufs=1, space="PSUM") as ps_l:

            # ---- consolidated input DMAs ----
            wq = pp.tile([128, 8 * 768], F32R, name="wq")
            nc.sync.dma_start(out=wq,
                              in_=wqkv_d.rearrange("(a b) c -> b a c", b=128))
            wout = pp.tile([128, 2048], F32R, name="wout")
            nc.sync.dma_start(out=wout,
                              in_=wout_d.rearrange("a b c -> b a c"))
            masks = pp.tile([128, 2048], BF16, name="masks")
            nc.sync.dma_start(out=masks,
                              in_=masks_d.rearrange("a b c -> b a c"))
            consts = pp.tile([128, 66], BF16, name="consts")
            nc.sync.dma_start(out=consts, in_=consts_d)
            ident = consts[:, 0:64]   # eye(64) stacked twice on partitions
            ones = consts[:, 64:66]
            xt = pp.tile([128, 8 * L], F32R, name="xt")
            nc.sync.dma_start(out=xt,
                              in_=xt_d.rearrange("(a b) c -> b a c", b=128))

            # persistent per-unit tensors
            qT = [pp.tile([128, L], F32R, name=f"qT{u}") for u in range(2)]
            kT = [pp.tile([128, L], F32R, name=f"kT{u}") for u in range(2)]
            vTb = [pp.tile([128, L], BF16, name=f"vTb{u}") for u in range(2)]
            vsb = [pp.tile([128, 2048], BF16, name=f"vsb{u}") for u in range(2)]
            outT = [pp.tile([128, L], F32R, name=f"outT{u}") for u in range(2)]

            for u in range(2):
                # ---- phase 1: qkvT for this unit ----
                for ci, dest in ((0, qT[u]), (1, kT[u]), (2, vTb[u])):
                    cof = u * 384 + ci * 128
                    for r in range(4):
                        acc = ps_mm.tile([128, 512], F32, name=f"qkv{u}{ci}{r}",
                                         tag="mm512")
                        for d in range(8):
                            nc.tensor.matmul(
                                acc, wq[:, d * 768 + cof:d * 768 + cof + 128],
                                xt[:, d * L + r * 512:d * L + (r + 1) * 512],
                                start=(d == 0), stop=(d == 7))
                        nc.vector.tensor_copy(
                            dest[:, r * 512:(r + 1) * 512], acc)

                # ---- V^T -> V transposes (bf16, via PE) ----
                for h2 in range(2):
                    st = ps_stage.tile([128, 1024], BF16, name=f"vt{u}{h2}",
                                       tag="stage")
                    for j in range(16):
                        nc.tensor.transpose(
                            st[:, j * 64:(j + 1) * 64],
                            vTb[u][h2 * 64:(h2 + 1) * 64,
                                   j * 128:(j + 1) * 128],
                            ident[h2 * 64:(h2 + 1) * 64, :])
                    nc.vector.tensor_copy(
                        vsb[u][:, h2 * 1024:(h2 + 1) * 1024], st)

                # ---- phase 2: attention ----
                for I in range(NQB):
                    nj = 4 * (I + 1)
                    out_ps = ps_av.tile([128, 512], F32, name=f"av{u}{I}",
                                        tag="av")
                    l_ps = ps_l.tile([128, 512], F32, name=f"l{u}{I}", tag="l")
                    qs = slice(I * 512, (I + 1) * 512)
                    for j in range(nj):
                        ks = slice(j * 128, (j + 1) * 128)
                        st = ps_stage.tile([128, 1024], F32,
                                           name=f"st{u}{I}{j}", tag="stage")
                        nc.tensor.matmul(st[:, 0:512], kT[u][0:64, ks],
                                         qT[u][0:64, qs], start=True,
                                         stop=True, skip_group_check=True)
                        nc.tensor.matmul(st[:, 512:1024], kT[u][64:128, ks],
                                         qT[u][64:128, qs], start=True,
                                         stop=True, skip_group_check=True)
                        pt = wp.tile([128, 1024], BF16, name=f"pt{u}{I}{j}",
                                     tag="pt", bufs=3)
                        nc.scalar.activation(pt, st, EXP, scale=0.125)
                        if j >= 4 * I:
                            m = j - 4 * I
                            ms = slice(m * 512, (m + 1) * 512)
                            nc.vector.tensor_mul(pt[:, 0:512], pt[:, 0:512],
                                                 masks[:, ms])
                            nc.vector.tensor_mul(pt[:, 512:1024],
                                                 pt[:, 512:1024], masks[:, ms])
                        fl = (j == 0)
                        ll = (j == nj - 1)
                        nc.tensor.matmul(out_ps[0:64, :],
                                         vsb[u][:, j * 64:(j + 1) * 64],
                                         pt[:, 0:512], start=fl, stop=ll,
                                         tile_position=(0, 0),
                                         skip_group_check=True)
                        nc.tensor.matmul(out_ps[64:128, :],
                                         vsb[u][:, 1024 + j * 64:
                                                1024 + (j + 1) * 64],
                                         pt[:, 512:1024], start=fl, stop=ll,
                                         tile_position=(0, 64),
                                         skip_group_check=True)
                        nc.tensor.matmul(l_ps[0:1, :], ones[:, 0:1],
                                         pt[:, 0:512], start=fl, stop=ll,
                                         tile_position=(0, 0),
                                         skip_group_check=True)
                        nc.tensor.matmul(l_ps[32:33, :], ones[:, 1:2],
                                         pt[:, 512:1024], start=fl, stop=ll,
                                         tile_position=(0, 32),
                                         skip_group_check=True)
                    # finalize q-block: 1/l broadcast + normalize
                    rt = wp.tile([128, 512], F32, name=f"rt{u}{I}", tag="rt",
                                 bufs=2)
                    nc.vector.reciprocal(rt[0:1, :], l_ps[0:1, :])
                    nc.vector.reciprocal(rt[32:33, :], l_ps[32:33, :])
                    rt2 = wp.tile([1, 512], F32, name=f"rt2{u}{I}",
                                  tag="rt2", bufs=2)
                    nc.gpsimd.dma_start(out=rt2, in_=rt[32:33, :])
                    bc0 = wp.tile([64, 512], F32, name=f"bc0{u}{I}", tag="bc0",
                                  bufs=2)
                    bc1 = wp.tile([64, 512], F32, name=f"bc1{u}{I}", tag="bc1",
                                  bufs=2)
                    nc.gpsimd.partition_broadcast(bc0, rt[0:1, :], channels=64)
                    nc.gpsimd.partition_broadcast(bc1, rt2, channels=64)
                    nc.vector.tensor_mul(outT[u][0:64, qs], out_ps[0:64, :],
                                         bc0)
                    nc.vector.tensor_mul(outT[u][64:128, qs],
                                         out_ps[64:128, :], bc1)

            # ---- phase 3: output projection ----
            for qc in range(16):
                qs = slice(qc * 128, (qc + 1) * 128)
                ysb = wp.tile([128, 1024], F32, name=f"ys{qc}", tag="ysb",
                              bufs=2)
                for nck in range(2):
                    ns = slice(nck * 512, (nck + 1) * 512)
                    yps = ps_mm.tile([128, 512], F32, name=f"y{qc}{nck}",
                                     tag="mm512")
                    nc.tensor.matmul(yps, outT[0][:, qs],
                                     wout[:, ns], start=True, stop=False)
                    nc.tensor.matmul(yps, outT[1][:, qs],
                                     wout[:, 1024 + ns.start:1024 + ns.stop],
                                     start=False, stop=True)
                    nc.vector.tensor_copy(ysb[:, ns], yps)
                nc.sync.dma_start(out=y_d[qs, :], in_=ysb)

    nc.compile()
    return nc


def _host_inputs(x, w_qkv, w_out):
    """Build per-core input maps."""
    x = np.asarray(x, dtype=np.float32)
    w_qkv = np.asarray(w_qkv, dtype=np.float32)
    w_out = np.asarray(w_out, dtype=np.float32)

    xts = [np.ascontiguousarray(x[b].T) for b in range(B)]  # (D, L)

    # stair masks: mask_m[k_rel, q] = 1 if 128*m + k_rel <= q else 0
    masks = np.zeros((4, 128, 512), dtype=ml_dtypes.bfloat16)
    kk = np.arange(128)[:, None]
    qq = np.arange(512)[None, :]
    for m in range(4):
        masks[m] = (128 * m + kk <= qq).astype(ml_dtypes.bfloat16)
    consts = np.zeros((128, 66), dtype=ml_dtypes.bfloat16)
    consts[:, 0:64] = np.tile(np.eye(64), (2, 1)).astype(ml_dtypes.bfloat16)
    consts[:, 64:66] = 1.0

    in_maps = []
    for c in range(NCORES):
        b, g = divmod(c, 4)
        heads = [4 * g + i for i in range(4)]
        # wqkv_local: per unit u: [q(128) | k(128) | v(128)] for heads
        # (4g+2u, 4g+2u+1)
        cols = []
        for u in range(2):
            h0, h1 = heads[2 * u], heads[2 * u + 1]
            for part in range(3):  # q, k, v sections at offsets 0, D, 2D
                off = part * D
                cols.append(w_qkv[:, off + h0 * DH: off + (h0 + 1) * DH])
                cols.append(w_qkv[:, off + h1 * DH: off + (h1 + 1) * DH])
        wqkv_local = np.ascontiguousarray(np.concatenate(cols, axis=1))
        # wout_local[u]: rows for heads (4g+2u, 4g+2u+1) stacked [64+64, 1024]
        wo = np.zeros((2, 128, 1024), dtype=np.float32)
        for u in range(2):
            h0, h1 = heads[2 * u], heads[2 * u + 1]
            wo[u, 0:64] = w_out[h0 * DH:(h0 + 1) * DH, :]
            wo[u, 64:128] = w_out[h1 * DH:(h1 + 1) * DH, :]
        in_maps.append({
            "xt": xts[b],
            "wqkv": wqkv_local,
            "wout": wo,
            "masks": masks,
            "consts": consts,
        })
    return in_maps


def kernel(x, w_qkv, w_out):
    global LAST_RESULT
    if "nc" not in _CACHE:
        _CACHE["nc"] = _build()
    nc = _CACHE["nc"]
    in_maps = _host_inputs(x, w_qkv, w_out)
    res = run_bass_kernel_spmd(nc, in_maps, list(range(NCORES)))
    LAST_RESULT = res
    y = np.zeros((B, L, D), dtype=np.float32)
    for c in range(NCORES):
        y[c // 4] += res.results[c]["y"]
    return y


# revision 18
# speedup vs baseline: 6.8289x; 6.8289x over previous
"""Causal self-attention TRN2 kernel (B=2, L=2048, D=1024, H=16, dh=64).

Sharding: 8 cores = 2 batches x 4 head-groups. Core c handles batch c//4 and
heads [4g..4g+3] where g = c % 4, as two head-pairs ("units").

Per-core device program (SPMD, same program all cores, different data):
  phase 1: qkvT = W_local.T @ x_b   (fp32r, contraction on partitions)
  phase 2: per unit (head-pair), per 512-wide q-block:
           S^T chunks [128k x 512q] via fp32r K=64 row-split matmul pairs,
           exp on ScalarE (PSUM->SBUF, bf16 out, scale=1/8, no max-sub),
           causal stair masks via bf16 DVE multiplies,
           AV (col-packed M=64 pair) + row-sums l (M=1 pair) in bf16,
           normalize by 1/l (DVE reciprocal + gpsimd partition_broadcast).
  phase 3: y_partial = outT.T @ w_out_local (fp32r), DMA to DRAM.
Host: y[b] = sum of the 4 partial outputs for batch b.
"""

import numpy as np
import ml_dtypes

import concourse.bass as bass
import concourse.mybir as mybir
from concourse import bacc
import concourse.tile as tile
from concourse.bass_utils import run_bass_kernel_spmd

F32 = mybir.dt.float32
F32R = mybir.dt.float32r
BF16 = mybir.dt.bfloat16
FP16 = mybir.dt.float16
EXP = mybir.ActivationFunctionType.Exp

B, L, D = 2, 2048, 1024
H, DH = 16, 64
NCORES = 8
NQB = L // 512          # q-blocks per sequence (4)

_CACHE = {}
LAST_RESULT = None      # BassKernelResults of the most recent run (for test.py)


def _build():
    nc = bacc.Bacc("TRN2", target_bir_lowering=False, debug=False,
                   num_devices=NCORES)

    xt_d = nc.dram_tensor("xt", [D, L], F32R, kind="ExternalInput").ap()
    wqkv_d = nc.dram_tensor("wqkv", [D, 768], F32R, kind="ExternalInput").ap()
    wout_d = nc.dram_tensor("wout", [2, 128, 1024], F32R,
                            kind="ExternalInput").ap()
    masks_d = nc.dram_tensor("masks", [4, 128, 512], FP16,
                             kind="ExternalInput").ap()
    consts_d = nc.dram_tensor("consts", [128, 66], FP16,
                              kind="ExternalInput").ap()
    y_d = nc.dram_tensor("y", [L, D], F32, kind="ExternalOutput").ap()

    with tile.TileContext(nc) as tc:
        with tc.tile_pool(name="persist", bufs=1) as pp, \
             tc.tile_pool(name="work", bufs=1) as wp, \
             tc.tile_pool(name="psstage", bufs=3, space="PSUM") as ps_stage, \
             tc.tile_pool(name="psav", bufs=1, space="PSUM") as ps_av, \
             tc.tile_pool(name="psl", bufs=1, space="PSUM") as ps_l:

            # ---- consolidated input DMAs ----
            wq = pp.tile([128, 8 * 768], F32R, name="wq")
            wq_dst = wq.rearrange("p (d c) -> p d c", d=8)
            wq_src = wqkv_d.rearrange("(a b) c -> b a c", b=128)
            xt = pp.tile([128, 8 * L], F32R, name="xt")
            xt_dst = xt.rearrange("p (d l) -> p d l", d=8)
            xt_src = xt_d.rearrange("(a b) c -> b a c", b=128)

            def load_wq(u, ci):
                cof = u * 384 + ci * 128
                nc.sync.dma_start(out=wq_dst[:, :, cof:cof + 128],
                                  in_=wq_src[:, :, cof:cof + 128])

            def load_xt(q, dh):
                ds_ = slice(dh * 4, (dh + 1) * 4)
                nc.sync.dma_start(
                    out=xt_dst[:, ds_, q * 512:(q + 1) * 512],
                    in_=xt_src[:, ds_, q * 512:(q + 1) * 512])

            load_wq(0, 0)
            load_xt(0, 0)
            load_xt(0, 1)
            load_xt(1, 0)
            load_xt(1, 1)
            load_wq(0, 1)
            load_wq(0, 2)
            load_xt(2, 0)
            load_xt(2, 1)
            load_xt(3, 0)
            load_xt(3, 1)
            load_wq(1, 0)
            load_wq(1, 1)
            load_wq(1, 2)
            masks = pp.tile([128, 2048], FP16, name="masks")
            nc.sync.dma_start(out=masks,
                              in_=masks_d.rearrange("a b c -> b a c"))
            consts = pp.tile([128, 66], FP16, name="consts")
            nc.sync.dma_start(out=consts, in_=consts_d)
            ident = consts[:, 0:64]   # eye(64) stacked twice on partitions
            ones = consts[:, 64:66]
            wout = pp.tile([128, 2048], F32R, name="wout")
            nc.sync.dma_start(out=wout,
                              in_=wout_d.rearrange("a b c -> b a c"))

            # persistent per-unit tensors
            qT = [pp.tile([128, L], F32R, name=f"qT{u}") for u in range(2)]
            kT = [pp.tile([128, L], F32R, name=f"kT{u}") for u in range(2)]
            vTb = [pp.tile([128, L], FP16, name=f"vTb{u}") for u in range(2)]
            vsb = [pp.tile([128, 2048], FP16, name=f"vsb{u}") for u in range(2)]
            outT = [pp.tile([128, L], F32R, name=f"outT{u}") for u in range(2)]

            def unit_qkv(u):
                # ---- phase 1: qkvT for this unit ----
                for ci, dest in ((0, qT[u]), (1, kT[u]), (2, vTb[u])):
                    cof = u * 384 + ci * 128
                    for rp in range(2):
                        acc = ps_stage.tile([128, 1024], F32,
                                            name=f"qkv{u}{ci}{rp}", tag="stage")
                        for half in range(2):
                            r = rp * 2 + half
                            hs = slice(half * 512, (half + 1) * 512)
                            for d in (0, 1, 2, 3, 4, 5, 6, 7):
                                nc.tensor.matmul(
                                    acc[:, hs],
                                    wq[:, d * 768 + cof:d * 768 + cof + 128],
                                    xt[:, d * L + r * 512:
                                       d * L + (r + 1) * 512],
                                    start=(d == 0), stop=(d == 7),
                                    skip_group_check=True)
                        nc.vector.tensor_copy(
                            dest[:, rp * 1024:(rp + 1) * 1024], acc)

                # ---- V^T -> V transposes (bf16, via PE) ----
                for h2 in range(2):
                    st = ps_stage.tile([128, 1024], FP16, name=f"vt{u}{h2}",
                                       tag="stage")
                    for j in range(16):
                        nc.tensor.transpose(
                            st[:, j * 64:(j + 1) * 64],
                            vTb[u][h2 * 64:(h2 + 1) * 64,
                                   j * 128:(j + 1) * 128],
                            ident[h2 * 64:(h2 + 1) * 64, :])
                    nc.vector.tensor_copy(
                        vsb[u][:, h2 * 1024:(h2 + 1) * 1024], st)

            def attn_qblock(u, I):
                    nj = 4 * (I + 1)
                    out_ps = ps_av.tile([128, 512], F32, name=f"av{u}{I}",
                                        tag="av")
                    l_ps = ps_l.tile([128, 512], F32, name=f"l{u}{I}", tag="l")
                    qs = slice(I * 512, (I + 1) * 512)
                    for j in range(nj):
                        ks = slice(j * 128, (j + 1) * 128)
                        m = j - 4 * I
                        qlo = max(m, 0) * 128   # first valid q_rel
                        w = 512 - qlo           # valid width
                        qv = slice(qs.start + qlo, qs.stop)
                        st = ps_stage.tile([128, 1024], F32,
                                           name=f"st{u}{I}{j}", tag="stage")
                        nc.tensor.matmul(st[:, qlo:512], kT[u][0:64, ks],
                                         qT[u][0:64, qv], start=True,
                                         stop=True, skip_group_check=True)
                        nc.tensor.matmul(st[:, 512 + qlo:1024],
                                         kT[u][64:128, ks],
                                         qT[u][64:128, qv], start=True,
                                         stop=True, skip_group_check=True)
                        pt = wp.tile([128, 1024], FP16, name=f"pt{u}{I}{j}",
                                     tag="pt", bufs=4)
                        if qlo == 0:
                            nc.scalar.activation(pt, st, EXP, scale=0.125)
                        else:
                            nc.scalar.activation(pt[:, qlo:512],
                                                 st[:, qlo:512], EXP,
                                                 scale=0.125)
                            nc.scalar.activation(pt[:, 512 + qlo:1024],
                                                 st[:, 512 + qlo:1024], EXP,
                                                 scale=0.125)
                        if m >= 0:
                            ms = slice(m * 512 + qlo, (m + 1) * 512)
                            nc.vector.tensor_mul(pt[:, qlo:512],
                                                 pt[:, qlo:512], masks[:, ms])
                            nc.vector.tensor_mul(pt[:, 512 + qlo:1024],
                                                 pt[:, 512 + qlo:1024],
                                                 masks[:, ms])
                        fl = (j == 0)
                        ll = (j == nj - 1)
                        nc.tensor.matmul(out_ps[0:64, qlo:512],
                                         vsb[u][:, j * 64:(j + 1) * 64],
                                         pt[:, qlo:512], start=fl, stop=ll,
                                         tile_position=(0, 0),
                                         skip_group_check=True)
                        nc.tensor.matmul(out_ps[64:128, qlo:512],
                                         vsb[u][:, 1024 + j * 64:
                                                1024 + (j + 1) * 64],
                                         pt[:, 512 + qlo:1024], start=fl,
                                         stop=ll, tile_position=(0, 64),
                                         skip_group_check=True)
                        nc.tensor.matmul(l_ps[0:1, qlo:512], ones[:, 0:1],
                                         pt[:, qlo:512], start=fl, stop=ll,
                                         tile_position=(0, 0),
                                         skip_group_check=True)
                        nc.tensor.matmul(l_ps[32:33, qlo:512], ones[:, 1:2],
                                         pt[:, 512 + qlo:1024], start=fl,
                                         stop=ll, tile_position=(0, 32),
                                         skip_group_check=True)
                    # finalize q-block: 1/l broadcast + normalize
                    rt = wp.tile([128, 512], F32, name=f"rt{u}{I}", tag="rt",
                                 bufs=2)
                    nc.vector.reciprocal(rt[0:1, :], l_ps[0:1, :])
                    nc.vector.reciprocal(rt[32:33, :], l_ps[32:33, :])
                    rt2 = wp.tile([1, 512], F32, name=f"rt2{u}{I}",
                                  tag="rt2", bufs=2)
                    nc.gpsimd.dma_start(out=rt2, in_=rt[32:33, :])
                    bc0 = wp.tile([64, 512], F32, name=f"bc0{u}{I}", tag="bc0",
                                  bufs=2)
                    bc1 = wp.tile([64, 512], F32, name=f"bc1{u}{I}", tag="bc1",
                                  bufs=2)
                    nc.gpsimd.partition_broadcast(bc0, rt[0:1, :], channels=64)
                    nc.gpsimd.partition_broadcast(bc1, rt2[0:1, :],
                                                  channels=64)
                    nc.vector.tensor_mul(outT[u][0:64, qs], out_ps[0:64, :],
                                         bc0)
                    nc.vector.tensor_mul(outT[u][64:128, qs],
                                         out_ps[64:128, :], bc1)

            def outproj_qblock(I):
              for qc in range(4 * I, 4 * I + 4):
                qs = slice(qc * 128, (qc + 1) * 128)
                ysb = wp.tile([128, 1024], F32, name=f"ys{qc}", tag="ysb",
                              bufs=4)
                yps = ps_stage.tile([128, 1024], F32, name=f"y{qc}",
                                    tag="stage")
                for nck in range(2):
                    ns = slice(nck * 512, (nck + 1) * 512)
                    nc.tensor.matmul(yps[:, ns], outT[0][:, qs],
                                     wout[:, ns], start=True, stop=False,
                                     skip_group_check=True)
                    nc.tensor.matmul(yps[:, ns], outT[1][:, qs],
                                     wout[:, 1024 + ns.start:1024 + ns.stop],
                                     start=False, stop=True,
                                     skip_group_check=True)
                nc.vector.tensor_copy(ysb, yps)
                nc.sync.dma_start(out=y_d[qs, :], in_=ysb)

            unit_qkv(0)
            for I in range(NQB):
                attn_qblock(0, I)
            unit_qkv(1)
            for I in range(NQB):
                attn_qblock(1, I)
                if I > 0:
                    outproj_qblock(I - 1)
            outproj_qblock(NQB - 1)

    nc.compile()
    return nc


def _host_inputs(x, w_qkv, w_out):
    """Build per-core input maps."""
    x = np.asarray(x, dtype=np.float32)
    w_qkv = np.asarray(w_qkv, dtype=np.float32)
    w_out = np.asarray(w_out, dtype=np.float32)

    xts = [np.ascontiguousarray(x[b].T) for b in range(B)]  # (D, L)

    # stair masks: mask_m[k_rel, q] = 1 if 128*m + k_rel <= q else 0
    masks = np.zeros((4, 128, 512), dtype=np.float16)
    kk = np.arange(128)[:, None]
    qq = np.arange(512)[None, :]
    for m in range(4):
        masks[m] = (128 * m + kk <= qq).astype(np.float16)
    consts = np.zeros((128, 66), dtype=np.float16)
    consts[:, 0:64] = np.tile(np.eye(64), (2, 1)).astype(np.float16)
    consts[:, 64:66] = 1.0

    in_maps = []
    for c in range(NCORES):
        b, g = divmod(c, 4)
        heads = [4 * g + i for i in range(4)]
        # wqkv_local: per unit u: [q(128) | k(128) | v(128)] for heads
        # (4g+2u, 4g+2u+1)
        cols = []
        for u in range(2):
            h0, h1 = heads[2 * u], heads[2 * u + 1]
            for part in range(3):  # q, k, v sections at offsets 0, D, 2D
                off = part * D
                cols.append(w_qkv[:, off + h0 * DH: off + (h0 + 1) * DH])
                cols.append(w_qkv[:, off + h1 * DH: off + (h1 + 1) * DH])
        wqkv_local = np.ascontiguousarray(np.concatenate(cols, axis=1))
        # wout_local[u]: rows for heads (4g+2u, 4g+2u+1) stacked [64+64, 1024]
        wo = np.zeros((2, 128, 1024), dtype=np.float32)
        for u in range(2):
            h0, h1 = heads[2 * u], heads[2 * u + 1]
            wo[u, 0:64] = w_out[h0 * DH:(h0 + 1) * DH, :]
            wo[u, 64:128] = w_out[h1 * DH:(h1 + 1) * DH, :]
        in_maps.append({
            "xt": xts[b],
            "wqkv": wqkv_local,
            "wout": wo,
            "masks": masks,
            "consts": consts,
        })
    return in_maps


def kernel(x, w_qkv, w_out):
    global LAST_RESULT
    if "nc" not in _CACHE:
        _CACHE["nc"] = _build()
    nc = _CACHE["nc"]
    in_maps = _host_inputs(x, w_qkv, w_out)
    res = run_bass_kernel_spmd(nc, in_maps, list(range(NCORES)))
    LAST_RESULT = res
    y = np.zeros((B, L, D), dtype=np.float32)
    for c in range(NCORES):
        y[c // 4] += res.results[c]["y"]
    return y
